# revision 2
# baseline (speedup 1.0000x reference)
"""Trainium2 Bass kernel v10 for nn_Decoder (512-step LSTM scan, B=256, F=256).

Data-parallel over batch across 8 NeuronCores (32 batch/core). Steps 1..511
run on device (step 0 + initial_layer on host); W_ih+W_hh fold into one
combined weight since the LSTM input equals the previous hidden state.

v9 = v8 (SBUF output ring, 3 PSUM gate banks Y=[g] X=[f,i] Z=[o], selector
bias matmuls, bulk chunked output DMA) + t12 packing: tanh(g) is written
into the carried state tile next to c_{t-1}, so ONE [128,128] DVE mul
computes [SF*c | SI*TG] — the per-step DVE chain is 3 ops instead of 4.
On this stack per-instruction fixed costs (~1-2us on ACT/DVE) dominate,
so the serial chain's op count is what matters.
"""
import sys

sys.path.insert(0, "/opt/trn_rl_repo")

import numpy as np

SEQ_LEN = 512
B, L, F = 256, 128, 256
NCORES = 8
BS = B // NCORES  # 32 batch per core
CHUNK = 64        # ring slots per output DMA chunk

_CACHE = {}
VERSION = 11


def _sigmoid(x):
    out = np.empty_like(x)
    pos = x >= 0
    out[pos] = 1.0 / (1.0 + np.exp(-x[pos]))
    e = np.exp(x[~pos])
    out[~pos] = e / (1.0 + e)
    return out


def _build(steps, repeat=1):
    """Build + schedule the per-core Bass program (same program all cores)."""
    import concourse.mybir as mybir
    import concourse.tile as tile
    from concourse import bacc

    f32 = mybir.dt.float32
    AF = mybir.ActivationFunctionType

    n_slots = steps + 1
    n_chunks = (n_slots + CHUNK - 1) // CHUNK

    nc = bacc.Bacc("TRN2", target_bir_lowering=False, debug=False)

    h0_d = nc.dram_tensor("h0", [128, 2 * BS], f32, kind="ExternalInput")
    c0_d = nc.dram_tensor("c0", [128, 2 * BS], f32, kind="ExternalInput")
    wst_d = nc.dram_tensor("wst", [128, 16 * 128], f32, kind="ExternalInput")
    bselY_d = nc.dram_tensor("bselY", [2, 128], f32, kind="ExternalInput")
    bselX_d = nc.dram_tensor("bselX", [4, 128], f32, kind="ExternalInput")
    bselZ_d = nc.dram_tensor("bselZ", [2, 128], f32, kind="ExternalInput")
    selY_d = nc.dram_tensor("selY", [2, 2 * BS], f32, kind="ExternalInput")
    selX_d = nc.dram_tensor("selX", [4, 4 * BS], f32, kind="ExternalInput")
    selZ_d = nc.dram_tensor("selZ", [2, 2 * BS], f32, kind="ExternalInput")
    stag_d = nc.dram_tensor("stag", [VERSION, steps, repeat], f32,
                            kind="ExternalInput")
    outs_d = nc.dram_tensor("outs", [n_chunks, 128, CHUNK * 2 * BS], f32,
                            kind="ExternalOutput")

    with tile.TileContext(nc) as tc:
        with tc.tile_pool(name="const", bufs=1) as cpool, \
             tc.tile_pool(name="ring", bufs=1) as rpool, \
             tc.tile_pool(name="state", bufs=3) as spool, \
             tc.tile_pool(name="work", bufs=2) as wpool, \
             tc.tile_pool(name="psY", bufs=2, space="PSUM") as psYp, \
             tc.tile_pool(name="psX", bufs=2, space="PSUM") as psXp, \
             tc.tile_pool(name="psZ", bufs=2, space="PSUM") as psZp:

            wst_sb = cpool.tile([128, 16 * 128], f32)
            nc.gpsimd.dma_start(out=wst_sb[:], in_=wst_d.ap())
            bselY_sb = cpool.tile([2, 128], f32)
            nc.gpsimd.dma_start(out=bselY_sb[:], in_=bselY_d.ap())
            bselX_sb = cpool.tile([4, 128], f32)
            nc.gpsimd.dma_start(out=bselX_sb[:], in_=bselX_d.ap())
            bselZ_sb = cpool.tile([2, 128], f32)
            nc.gpsimd.dma_start(out=bselZ_sb[:], in_=bselZ_d.ap())
            selY_sb = cpool.tile([2, 2 * BS], f32)
            nc.gpsimd.dma_start(out=selY_sb[:], in_=selY_d.ap())
            selX_sb = cpool.tile([4, 4 * BS], f32)
            nc.gpsimd.dma_start(out=selX_sb[:], in_=selX_d.ap())
            selZ_sb = cpool.tile([2, 2 * BS], f32)
            nc.gpsimd.dma_start(out=selZ_sb[:], in_=selZ_d.ap())
            stag_sb = cpool.tile([1, 1], f32)
            nc.sync.dma_start(out=stag_sb[:], in_=stag_d.ap()[0:1, 0:1, 0])

            W = 2 * BS  # 64 cols per ring slot

            def scan_body():
                ring = [rpool.tile([128, CHUNK * W], f32, tag=f"ring{c}",
                                   name=f"ring{c}")
                        for c in range(n_chunks)]

                def slot(t):
                    c, s = divmod(t, CHUNK)
                    return ring[c][:, s * W:(s + 1) * W]

                nc.sync.dma_start(out=slot(0), in_=h0_d.ap())
                # carried state tile: [c_{t-1} | tanh(g_t)] (t12 packing)
                ct_first = spool.tile([128, 2 * W], f32, tag="ct")
                nc.sync.dma_start(out=ct_first[:, 0:W], in_=c0_d.ap())
                ct_cur = ct_first

                for t in range(1, steps + 1):
                    h_prev = slot(t - 1)

                    # bias matmuls first: they don't read h, so they fill
                    # the PE queue during the previous step's tail
                    psY = psYp.tile([128, W], f32, tag="psY")
                    nc.tensor.matmul(psY[:], lhsT=bselY_sb[:], rhs=selY_sb[:],
                                     start=True, stop=False)
                    psX = psXp.tile([128, 2 * W], f32, tag="psX")
                    nc.tensor.matmul(psX[:], lhsT=bselX_sb[:], rhs=selX_sb[:],
                                     start=True, stop=False)
                    psZ = psZp.tile([128, W], f32, tag="psZ")
                    nc.tensor.matmul(psZ[:], lhsT=bselZ_sb[:], rhs=selZ_sb[:],
                                     start=True, stop=False)

                    # weight matmuls k-outer: all k=0 MMs need only the k=0
                    # half of h (written first by the split tail below)
                    for k in range(2):
                        for j in range(2):
                            nc.tensor.matmul(
                                psY[:, BS * j:BS * (j + 1)],
                                lhsT=wst_sb[:, (2 * j + k) * 128:(2 * j + k + 1) * 128],
                                rhs=h_prev[:, BS * k:BS * (k + 1)],
                                start=False, stop=(j == 1 and k == 1))
                    # tanh(g) lands NEXT TO c_{t-1} in the carried state tile
                    nc.scalar.activation(ct_cur[:, W:2 * W], psY[:], AF.Tanh)

                    for k in range(2):
                        for gg in range(2):
                            for j in range(2):
                                m = 2 + gg * 2 + j
                                nc.tensor.matmul(
                                    psX[:, BS * (2 * gg + j):BS * (2 * gg + j + 1)],
                                    lhsT=wst_sb[:, (2 * m + k) * 128:(2 * m + k + 1) * 128],
                                    rhs=h_prev[:, BS * k:BS * (k + 1)],
                                    start=False,
                                    stop=(k == 1 and gg == 1 and j == 1))
                    SFI = wpool.tile([128, 2 * W], f32, tag="SFI")
                    nc.scalar.activation(SFI[:], psX[:], AF.Sigmoid)

                    for k in range(2):
                        for j in range(2):
                            m = 6 + j
                            nc.tensor.matmul(
                                psZ[:, BS * j:BS * (j + 1)],
                                lhsT=wst_sb[:, (2 * m + k) * 128:(2 * m + k + 1) * 128],
                                rhs=h_prev[:, BS * k:BS * (k + 1)],
                                start=False, stop=(k == 1 and j == 1))
                    SO = wpool.tile([128, W], f32, tag="SO")
                    nc.scalar.activation(SO[:], psZ[:], AF.Sigmoid)

                    # t12 = [SF*c | SI*TG] in ONE [128,128] mul
                    t12 = wpool.tile([128, 2 * W], f32, tag="t12")
                    nc.vector.tensor_mul(t12[:], SFI[:], ct_cur[:])
                    # k-split tail: finish the j=0 half of c/h first so the
                    # next step's k=0 matmuls can start while j=1 finishes
                    ct_new = spool.tile([128, 2 * W], f32, tag="ct")
                    nc.vector.tensor_add(ct_new[:, 0:BS], t12[:, 0:BS],
                                         t12[:, W:W + BS])
                    nc.vector.tensor_add(ct_new[:, BS:W], t12[:, BS:W],
                                         t12[:, W + BS:2 * W])
                    tch = wpool.tile([128, W], f32, tag="tch")
                    nc.scalar.activation(tch[:, 0:BS], ct_new[:, 0:BS],
                                         AF.Tanh)
                    nc.vector.tensor_mul(slot(t)[:, 0:BS], SO[:, 0:BS],
                                         tch[:, 0:BS])
                    nc.scalar.activation(tch[:, BS:W], ct_new[:, BS:W],
                                         AF.Tanh)
                    nc.vector.tensor_mul(slot(t)[:, BS:W], SO[:, BS:W],
                                         tch[:, BS:W])

                    ct_cur = ct_new

                    # chunk complete -> bulk DMA off the critical path
                    if t % CHUNK == CHUNK - 1 or t == steps:
                        c = t // CHUNK
                        nc.sync.dma_start(out=outs_d.ap()[c], in_=ring[c][:])

            if repeat == 1:
                scan_body()
            else:
                with tc.For_i(0, repeat, 1):
                    scan_body()

    nc.compile()
    return nc


def _get_nc(steps, repeat=1):
    key = (steps, repeat)
    if key not in _CACHE:
        _CACHE[key] = _build(steps, repeat)
    return _CACHE[key]


def _host_prep(x, last_feat, Wi, bi, W_ih, W_hh, b_ih, b_hh):
    x = np.asarray(x, np.float32)
    last_feat = np.asarray(last_feat, np.float32)
    Wi = np.asarray(Wi, np.float32); bi = np.asarray(bi, np.float32)
    W_ih = np.asarray(W_ih, np.float32); W_hh = np.asarray(W_hh, np.float32)
    b_ih = np.asarray(b_ih, np.float32); b_hh = np.asarray(b_hh, np.float32)

    z = x[0] @ Wi.T + bi                       # [B, F]
    init = np.where(z > 0, z, np.expm1(z)).astype(np.float32)  # elu

    bsum = b_ih + b_hh
    g0 = last_feat @ W_ih.T + init @ W_hh.T + bsum   # [B, 4F] order i,f,g,o
    i0, f0, g0g, o0 = (g0[:, 0:F], g0[:, F:2*F], g0[:, 2*F:3*F], g0[:, 3*F:4*F])
    c1 = _sigmoid(f0) * init + _sigmoid(i0) * np.tanh(g0g)
    h1 = (_sigmoid(o0) * np.tanh(c1)).astype(np.float32)
    c1 = c1.astype(np.float32)

    Wc = (W_ih + W_hh).astype(np.float32)       # [4F, F] gate order i,f,g,o
    # MM-emission gate order: Y=g(2), X=f(1),i(0), Z=o(3)
    gates = [2, 1, 0, 3]
    wst = np.empty((128, 16 * 128), np.float32)
    for gi, g in enumerate(gates):
        Wg = Wc[g * F:(g + 1) * F]              # [256, 256]
        for j in range(2):
            for k in range(2):
                m = gi * 2 + j
                wst[:, (2 * m + k) * 128:(2 * m + k + 1) * 128] = \
                    Wg[128 * j:128 * (j + 1), 128 * k:128 * (k + 1)].T
    bY = bsum[2 * F:3 * F]
    bF = bsum[F:2 * F]
    bI = bsum[0:F]
    bZ = bsum[3 * F:4 * F]
    bselY = bY.reshape(2, 128).astype(np.float32)
    bselX = np.concatenate([bF, bI]).reshape(4, 128).astype(np.float32)
    bselZ = bZ.reshape(2, 128).astype(np.float32)
    return h1, c1, wst, bselY, bselX, bselZ


def _to_fm(a):
    """[BS, F] batch-major -> [128, 2*BS] feature-major."""
    return np.ascontiguousarray(
        a.T.reshape(2, 128, BS).transpose(1, 0, 2).reshape(128, 2 * BS))


def _sel(q):
    s = np.zeros((q, q * BS), np.float32)
    for i in range(q):
        s[i, BS * i:BS * (i + 1)] = 1.0
    return s


def _in_maps(inputs, steps, repeat=1):
    h1, c1, wst, bselY, bselX, bselZ = _host_prep(
        inputs["x"], inputs["last_feat"], inputs["Wi"], inputs["bi"],
        inputs["W_ih"], inputs["W_hh"], inputs["b_ih"], inputs["b_hh"])
    stag = np.zeros((VERSION, steps, repeat), np.float32)
    maps = []
    for ci in range(NCORES):
        s = slice(ci * BS, (ci + 1) * BS)
        maps.append(dict(
            h0=_to_fm(h1[s]), c0=_to_fm(c1[s]), wst=wst,
            bselY=bselY, bselX=bselX, bselZ=bselZ,
            selY=_sel(2), selX=_sel(4), selZ=_sel(2), stag=stag))
    return maps, h1


def kernel(x, last_feat, Wi, bi, W_ih, W_hh, b_ih, b_hh, Wo, bo,
           _steps=SEQ_LEN - 1, _repeat=1):
    from concourse.bass_utils import run_bass_kernel_spmd

    inputs = dict(x=x, last_feat=last_feat, Wi=Wi, bi=bi, W_ih=W_ih,
                  W_hh=W_hh, b_ih=b_ih, b_hh=b_hh)
    in_maps, h1 = _in_maps(inputs, _steps, _repeat)
    nc = _get_nc(_steps, _repeat)
    res = run_bass_kernel_spmd(nc, in_maps, core_ids=list(range(NCORES)))

    n_slots = _steps + 1
    n_chunks = (n_slots + CHUNK - 1) // CHUNK
    outs = np.zeros((SEQ_LEN, B, F), np.float32)
    for ci, r in enumerate(res.results):
        arr = r["outs"].reshape(n_chunks, 128, CHUNK, 2, BS)
        # [c,p,ss,j,b] -> [c,ss,b,j,p] -> [slots, BS, F]
        arr = arr.transpose(0, 2, 4, 3, 1).reshape(n_chunks * CHUNK, BS, F)
        outs[:n_slots, ci * BS:(ci + 1) * BS, :] = arr[:n_slots]
    return np.ascontiguousarray(outs).reshape(B, SEQ_LEN, F)
